# revision 23
# baseline (speedup 1.0000x reference)
"""Trainium2 Bass kernel for CustomLoss:
    out = mean_{b,t} CE(logits[b,t,:], tgt[b,t]) + penalty
    CE   = logsumexp_V(logits) - logits[tgt]
    penalty = sum_b C(n_b, 2), n_b = #{t : sizes[b, argmax_V logits[b,t,:]] > 0}

Sharding: data-parallel over the 4096 (b,t) tokens -> 512 tokens/core on 8
NeuronCores. Each core streams its [512, 32000] logits shard through SBUF
once; DVE computes per-block max (one full-rate reduce pass), ACT computes
exp with fused free-axis accumulation (logsumexp). The argmax per token is
found hierarchically: max_index over the 128 block-maxes picks the winning
250-wide block, which is re-gathered from DRAM together with the matching
sizes block; a mask trick (blk == gmax -> mask * sizes -> reduce_max) then
yields sizes[argmax] without a second dependent gather. All Ln calls are
batched into one activation at the end so the ACT table set never thrashes
between Exp and Ln mid-stream. Per-core partial sums (sum of nll, count of
positive-size argmax tokens) are combined on host.
"""

from contextlib import ExitStack

import numpy as np

P = 128
V = 32000
B, T = 2, 2048
N_CORES = 8
TOK = (B * T) // N_CORES      # 512 tokens per core
NT = TOK // P                 # 4 token tiles of 128 partitions
W = 250                       # argmax block width
NB = V // W                   # 128 blocks per token row
# vocab chunks per tile; the last tile tapers off so the tail after the
# final DMA (last DVE reduce + last ACT exp) is short
CHUNKS = [
    [8000, 8000, 8000, 8000],
    [8000, 8000, 8000, 8000],
    [8000, 8000, 8000, 8000],
    [8000, 8000, 4000, 4000, 4000, 2000, 1000, 500, 500],
]
NCH_MAX = max(len(c) for c in CHUNKS)
ALPHA = 1.0

_NC_CACHE = {}


def _build_nc():
    """Build the single-core Bass program (identical on all 8 cores)."""
    import concourse.bacc as bacc
    import concourse.bass as bass
    import concourse.mybir as mybir
    import concourse.tile as tile

    f32 = mybir.dt.float32
    bf16 = mybir.dt.bfloat16
    i32 = mybir.dt.int32
    u32 = mybir.dt.uint32
    AF = mybir.ActivationFunctionType
    ALU = mybir.AluOpType
    AX = mybir.AxisListType

    nc = bacc.Bacc("TRN2", target_bir_lowering=False)
    logits = nc.declare_dram_parameter("logits", [TOK, V], f32, isOutput=False)
    # flat element index t*V + tgt[t], laid out [p, tile] (token = tt*128 + p)
    tgt_off = nc.declare_dram_parameter("tgt_off", [P, NT], i32, isOutput=False)
    sizes_r = nc.declare_dram_parameter("sizes_r", [1, V], f32, isOutput=False)
    out = nc.declare_dram_parameter("out", [1, 2], f32, isOutput=True)

    from concourse.tile_rust import add_dep_helper

    with tile.TileContext(nc) as tc, ExitStack() as ctx:
        lp = ctx.enter_context(tc.tile_pool(name="lp", bufs=5))
        ep = ctx.enter_context(tc.tile_pool(name="ep", bufs=1))
        sm = ctx.enter_context(tc.tile_pool(name="sm", bufs=4))
        cst = ctx.enter_context(tc.tile_pool(name="cst", bufs=1))
        pp = ctx.enter_context(tc.tile_pool(name="pp", bufs=1, space="PSUM"))

        # constants (vector/gpsimd; the sync engine's first instruction is
        # the first streaming DMA)
        ones = cst.tile([P, 1], f32)
        nc.vector.memset(ones[:], 1.0)
        # rowofft[tt][p] = tt*P*V + p*V as f32 (exact: flat indices < 2^24)
        rowofft = []
        for tt in range(NT):
            ri_ = cst.tile([P, 1], i32, tag=f"roi{tt}")
            nc.gpsimd.iota(
                ri_[:], pattern=[[1, 1]], base=tt * P * V, channel_multiplier=V
            )
            rf_ = cst.tile([P, 1], f32, tag=f"rof{tt}")
            nc.gpsimd.tensor_copy(rf_[:], ri_[:])
            rowofft.append(rf_)

        # tgt_logit gathers are spread one-per-tile through the stream (the
        # 512 random 4B reads would otherwise stall the first chunk DMAs)
        tgt_idx = cst.tile([P, NT], i32)
        nc.gpsimd.dma_start(tgt_idx[:], tgt_off[:, :])
        tgt_logit = cst.tile([P, NT], f32)

        tot_cols = cst.tile([P, NT], f32)
        szg_cols = cst.tile([P, NT], f32)
        m_cols = cst.tile([P, NT], f32)
        # ACT's per-chunk exp accumulators, all tiles side by side; the
        # unused tail columns of short tiles stay 0 from the memset
        sexp_all = cst.tile([P, NT * NCH_MAX], f32)
        nc.vector.memset(sexp_all[:], 0.0)
        tile_state = []
        reduce_insts = [[] for _ in range(NT)]

        for tt in range(NT):
            chunks = CHUNKS[tt]
            bmax = sm.tile([P, NB], f32, tag="bmax")
            voff = 0
            boff = 0
            for c, vc in enumerate(chunks):
                nblk = vc // W
                lt = lp.tile([P, vc], f32, tag="lt")
                nc.sync.dma_start(
                    lt[:], logits[tt * P : (tt + 1) * P, voff : voff + vc]
                )
                # per-block max in one full-rate pass: [P, nblk, W] -> [P, nblk]
                lt3 = lt[:].rearrange("p (b w) -> p b w", w=W)
                ri = nc.vector.tensor_reduce(
                    bmax[:, boff : boff + nblk], lt3, axis=AX.X, op=ALU.max
                )
                reduce_insts[tt].append(ri)
                # exp output is write-only scratch (the f32 accumulator is
                # the real result) - bf16 halves its SBUF footprint
                et = ep.tile([P, 8000], bf16, tag="et")
                cc = tt * NCH_MAX + c
                nc.scalar.activation(
                    et[:, :vc], lt[:], AF.Exp, accum_out=sexp_all[:, cc : cc + 1]
                )
                voff += vc
                boff += nblk

            # hierarchical argmax: winning block id via max_index over the
            # 128 block maxima, then re-gather that W-wide block from DRAM
            # together with the matching sizes block. gmax := mx8[:,0:1].
            mx8 = sm.tile([P, 8], f32, tag="mx8")
            nc.vector.max(mx8[:], bmax[:])
            gmax = mx8
            bix8 = sm.tile([P, 8], u32, tag="bix8")
            nc.vector.max_index(bix8[:], mx8[:], bmax[:])

            # index math (f32, exact below 2^24), split DVE/gpsimd so the
            # two gather offsets are ready as early as possible:
            #   soff = bix*W               (sizes block start)
            #   gsi  = bix*W + (tt*P+p)*V  (logits block start, flat)
            bixf = sm.tile([P, 1], f32, tag="bixf")
            nc.vector.tensor_copy(bixf[:], bix8[:, 0:1])
            sof = sm.tile([P, 1], f32, tag="sof")
            nc.vector.tensor_scalar(
                sof[:], bixf[:], float(W), None, op0=ALU.mult
            )
            soi = sm.tile([P, 1], i32, tag="soi")
            nc.vector.tensor_copy(soi[:], sof[:])
            gsf = sm.tile([P, 1], f32, tag="gsf")
            nc.gpsimd.tensor_scalar(
                gsf[:], bixf[:], float(W), rowofft[tt][:, 0:1],
                op0=ALU.mult, op1=ALU.add,
            )
            gsi = sm.tile([P, 1], i32, tag="gsi")
            nc.gpsimd.tensor_copy(gsi[:], gsf[:])

            blk = sm.tile([P, W], f32, tag="blk")
            nc.gpsimd.indirect_dma_start(
                out=blk[:],
                out_offset=None,
                in_=logits[:, :],
                in_offset=bass.IndirectOffsetOnAxis(ap=gsi[:, 0:1], axis=1),
            )
            sblk = sm.tile([P, W], f32, tag="sblk")
            nc.gpsimd.indirect_dma_start(
                out=sblk[:],
                out_offset=None,
                in_=sizes_r[:, :],
                in_offset=bass.IndirectOffsetOnAxis(ap=soi[:, 0:1], axis=1),
            )
            # tgt_logit gathers ride in tiles 0-2 (tile 2 carries tile 3's
            # too) so none sits in the last tile's critical tail segment
            for gt in ([tt] if tt < NT - 1 else []) + ([NT - 1] if tt == NT - 2 else []):
                nc.gpsimd.indirect_dma_start(
                    out=tgt_logit[:, gt : gt + 1],
                    out_offset=None,
                    in_=logits[:, :],
                    in_offset=bass.IndirectOffsetOnAxis(
                        ap=tgt_idx[:, gt : gt + 1], axis=1
                    ),
                )
            tile_state.append((gmax, blk, sblk))

        # m = (sizes[argmax] > 0) via mask trick: positions where
        # blk == gmax (>=1, the argmax), max of sizes there. Each tile's
        # gather-dependent DVE ops are pinned behind a later tile's chunk
        # reduce so they can never stall DVE mid-stream waiting on the
        # SWDGE gather (~6us flight after bix).
        last_szg = None
        for tt, (gmax, blk, sblk) in enumerate(tile_state):
            eqm = sm.tile([P, W], f32, tag="eqm")
            ei = nc.vector.tensor_scalar(
                eqm[:], blk[:], gmax[:, 0:1], None, op0=ALU.is_equal
            )
            if tt + 1 < NT:
                add_dep_helper(
                    ei.ins, reduce_insts[tt + 1][1].ins, sync=False,
                    reason="hide SWDGE gather latency behind streaming",
                )
            szm = sm.tile([P, W], f32, tag="szm")
            nc.vector.tensor_tensor(szm[:], eqm[:], sblk[:], op=ALU.mult)
            last_szg = nc.vector.tensor_reduce(
                szg_cols[:, tt : tt + 1], szm[:], axis=AX.X, op=ALU.max
            )
        # on DVE: keeps the final szg -> m -> acc -> matmul chain free of
        # cross-engine semaphore hops (GpSimd round trip cost ~0.6us here)
        nc.vector.tensor_scalar(
            m_cols[:], szg_cols[:], 0.0, None, op0=ALU.is_gt
        )

        # batched logsumexp tail: one 3D add-reduce collapses every tile's
        # exp accumulators, one Ln, one subtract. Pinned after the last
        # streaming reduce so DVE never waits on ACT mid-stream.
        sexp3 = sexp_all[:].rearrange("p (t c) -> p t c", c=NCH_MAX)
        ti = nc.vector.tensor_reduce(tot_cols[:], sexp3, axis=AX.X, op=ALU.add)
        add_dep_helper(
            ti.ins, reduce_insts[NT - 1][-1].ins, sync=False,
            reason="tot reduce only after all streaming DVE work",
        )
        lse_cols = cst.tile([P, NT], f32)
        nc.scalar.activation(lse_cols[:], tot_cols[:], AF.Ln)
        nll_cols = cst.tile([P, NT], f32)
        ni = nc.vector.tensor_tensor(
            nll_cols[:], lse_cols[:], tgt_logit[:], op=ALU.subtract
        )
        add_dep_helper(
            ni.ins, reduce_insts[NT - 1][-1].ins, sync=False,
            reason="nll only after all streaming DVE work",
        )

        # per-core partial sums: cross-partition reduce via matmul with ones
        acc = cst.tile([P, 2], f32)
        nc.vector.reduce_sum(acc[:, 0:1], nll_cols[:], axis=AX.X)
        nc.vector.reduce_sum(acc[:, 1:2], m_cols[:], axis=AX.X)
        ps = pp.tile([1, 2], f32)
        nc.tensor.matmul(ps[:], lhsT=ones[:], rhs=acc[:], start=True, stop=True)
        osb = cst.tile([1, 2], f32)
        nc.vector.tensor_copy(osb[:], ps[:])
        nc.sync.dma_start(out[:, :], osb[:])

    nc.finalize()
    return nc


def _get_nc():
    if "nc" not in _NC_CACHE:
        _NC_CACHE["nc"] = _build_nc()
    return _NC_CACHE["nc"]


def _make_in_maps(logits, tgt, sizes):
    logits = np.ascontiguousarray(np.asarray(logits, dtype=np.float32))
    tgt = np.asarray(tgt).astype(np.int64)
    sizes = np.ascontiguousarray(np.asarray(sizes, dtype=np.float32))

    flat_logits = logits.reshape(B * T, V)
    flat_tgt = tgt.reshape(B * T)

    in_maps = []
    for cid in range(N_CORES):
        lo = cid * TOK
        shard = flat_logits[lo : lo + TOK]                       # [TOK, V]
        toff = (np.arange(TOK, dtype=np.int64) * V + flat_tgt[lo : lo + TOK])
        toff = toff.astype(np.int32).reshape(NT, P).T.copy()     # [P, NT]
        b = (lo) // T
        assert (lo + TOK - 1) // T == b, "shard must not straddle batch rows"
        in_maps.append(
            {
                "logits": shard,
                "tgt_off": toff,
                "sizes_r": sizes[b].reshape(1, V),
            }
        )
    return in_maps


def _combine(results):
    nll_total = 0.0
    counts = np.zeros(B, dtype=np.float64)
    for cid, res in enumerate(results):
        o = np.asarray(res["out"], dtype=np.float64).reshape(2)
        nll_total += o[0]
        counts[(cid * TOK) // T] += o[1]
    ce = nll_total / (B * T)
    penalty = float(sum(n * (n - 1) / 2 for n in counts))
    return np.float32(ce + ALPHA * penalty)


def run(logits, tgt, sizes, trace=False):
    """Run the SPMD kernel on 8 cores. Returns (output_scalar, exec_time_ns)."""
    from concourse.bass_utils import run_bass_kernel_spmd

    nc = _get_nc()
    in_maps = _make_in_maps(logits, tgt, sizes)
    r = run_bass_kernel_spmd(nc, in_maps, list(range(N_CORES)), trace=trace)
    _NC_CACHE["last_result"] = r
    return _combine(r.results), r.exec_time_ns


def kernel(logits, tgt, sizes):
    out, _ = run(logits, tgt, sizes, trace=False)
    return out
